# revision 53
# baseline (speedup 1.0000x reference)
"""Sliding-window (banded) attention for nn_AttLayer on 8 Trainium2 NeuronCores.

Reference computation (per window-block n of 512 positions, 64 blocks over L=32768):
  q/k/v = 1x1-conv projections of x1 (512ch -> 256ch)
  energy[l, m] = (q_block[:, l] . k_window[:, m]) / 16   over a 1024-wide window
  attn = softmax(energy + log(band_mask + 1e-6)) * band_mask
  out  = relu(v_window @ attn^T) -> 1x1-conv (256 -> 512) + bias, masked

Sharding: 64 blocks split contiguously across 8 cores (8 blocks each). Each core
gets a zero-padded halo slice of x1 and computes its 4096 output columns.

Kernel strategy (per core, SPMD — all per-core variation is in the data):
  - Projections on PE in fp8e4 DoubleRow perf mode (0.5 cycles/row, two
    128-channel contraction tiles per instruction -> 4x MAC throughput).
    x1 and the projection weights are split host-side into e4m3 hi/lo pairs
    (W scaled by 64 to center the fp8 range; the 64^2 folds into the exp
    scale and 1/64 into Wo). Three accumulation chains (hi*hi + lo*hi +
    hi*lo) recover ~bf16-level accuracy at 0.75x the f32r cycle cost.
  - Everything downstream runs in fp16 (1 cycle/row like f32r but with no
    >=256 moving-width requirement, half the SBUF/DMA bytes of f32, and a
    10-bit mantissa -- ~10x less quantization error than bf16).
  - energy computed transposed: energyT[m, l] = k_chunk^T q (PE), over the
    EXACT per-chunk band intervals (2560 of 4096 window cols per block).
  - Band masking: affine_select on the otherwise-idle Pool/GPSIMD engine
    zeroes out-of-band exp values; sequence-edge padding handled with
    per-core 0/1 data vectors so the program stays SPMD.
  - exp on ScalarE (scale 2^-16 folds away the fp8 weight scaling),
    denominators via an all-ones fp16 matmul (column sums land replicated
    across partitions), reciprocal on VectorE.
  - AV on PE (fp16); softmax normalization + relu fused into the one
    PSUM->SBUF eviction (scalar_tensor_tensor max+mult) which also keeps
    the fp16 relu tile in range. Output projection in fp8 DoubleRow too
    (host-split Wo hi/lo x relu hi/lo, the relu split riding ACT+Pool
    slack); k evictions ride ACT (Identity + bias) so the DVE queue never
    gates energy. Final bias/mask applied on host.
  - Two-deep software pipeline: block b's colsum/AV/relu (tail_a) are
    emitted after block b+1's energy, and its output projection (tail_b)
    one block later still, so neither the softmax chain nor the relu
    hi/lo split chain ever stalls PE. The tail_b queue carries across the
    half boundary, covering the h0->h1 transition; half 1's x stream is
    prefetched during half 0 (xpool bufs=2).
  - Halo reuse between the two halves: half 1's first four vT tiles alias
    half 0's last four (identical x1 columns), and half 0's k overlap is
    stashed via an SBUF->SBUF DMA so half 1 skips its first k-group.
  - The final block takes a fused low-latency fp16 path (AV l-halves with
    reordered accumulation, ACT/DVE-alternating evictions, oc-pair output
    DMAs) because everything it exposes is pure drain-path latency.
"""

import numpy as np

NCORES = 8
L = 32768
CIN = 512
C = 256
BL = 512
HALF = 256
LC = L // NCORES              # 4096 positions per core
HALO = LC + 2 * HALF          # 4608
NBH = 2                       # halves per core
LH = LC // NBH                # 2048 positions per half
KSPAN = LH + 2 * HALF         # 2560 k/v positions per half
BPH = 4                       # blocks per half
WSCALE = 64.0                 # host-side fp8 scaling of Wq/Wk/Wv (and biases)
EXP_SCALE = (1.0 / 16.0) / (WSCALE * WSCALE)   # softmax scale / W-scaling^2

# Per m-chunk r (8 chunks of the 1024-wide window): EXACT valid l-interval
# (lo, width) within the block's 512 queries (fp16 has no min-width penalty).
INTERVALS = [
    (0, 128), (0, 256), (0, 384), (0, 512),
    (0, 512), (128, 384), (256, 256), (384, 128),
]
# accumulation order: r=3 covers the full [0,512) so it goes first (start=True)
AVORDER = [3, 4, 2, 5, 1, 6, 0, 7]


def _build_program():
    import concourse.mybir as mybir
    from concourse import bacc
    from concourse.tile import TileContext

    F32 = mybir.dt.float32
    F16 = mybir.dt.float16
    F8 = mybir.dt.float8e4
    BF16 = mybir.dt.bfloat16
    Alu = mybir.AluOpType
    Act = mybir.ActivationFunctionType
    PM = mybir.MatmulPerfMode.DoubleRow

    nc = bacc.Bacc()

    # x hi/lo fp8 halo slice in SBUF-tile order [pair p, c_within(128), hi/lo,
    # row j, pos] so ONE DMA per pair fills hi+lo and both j rows; weights in
    # DoubleRow pair layout [c_in_within_chunk(128), hi/lo, pair p, row j,
    # c_out] with global input channel 128*(2p+j) + c_in.
    xhl_d = nc.dram_tensor("xhl", [2, 128, 2, 2, HALO], F8, kind="ExternalInput")
    w_d = {}
    for kind in ("q", "k", "v"):
        w_d[kind] = nc.dram_tensor(
            f"w{kind}", [128, 2, 2, 2, C], F8, kind="ExternalInput")
    # f32 scalar blob: [bq0 bq1 bk0 bk1 padf0 padf1 padl0 padl1 | bvr(256)]
    cb32_d = nc.dram_tensor("cb32", [128, 264], F32, kind="ExternalInput")
    # f16 blob: [ones(128) | woT0(512) | woT1(512)]
    cb16_d = nc.dram_tensor("cb16", [128, 1152], F16, kind="ExternalInput")
    # fp8 hi/lo output-projection weights [c_in_within, hi/lo, cc, c_out]
    wo8_d = nc.dram_tensor("wo8", [128, 2, 2, CIN], F8, kind="ExternalInput")
    # c-major output: full channel g = 128*oc + c lives at [c, oc, :] — lets
    # one DMA carry several oc chunks (host re-interleaves)
    out_d = nc.dram_tensor("out", [128, 4, LC], F16, kind="ExternalOutput")

    with TileContext(nc) as tc:
        with (
            tc.tile_pool(name="consts", bufs=1) as consts,
            tc.tile_pool(name="xpool", bufs=2) as xpool,
            tc.tile_pool(name="qkv", bufs=1) as qkv,
            tc.tile_pool(name="ptp", bufs=2) as ptp,
            tc.tile_pool(name="sbo", bufs=4) as sbo,
            tc.tile_pool(name="pse", bufs=3, space="PSUM") as pse,
            tc.tile_pool(name="pss", bufs=1, space="PSUM") as pss,
            tc.tile_pool(name="psav", bufs=1, space="PSUM") as psav,
            tc.tile_pool(name="pso", bufs=2, space="PSUM") as pso,
        ):
            # warm the PE clock gate (HAM) during the initial DMA wait:
            # dummy bf16 matmuls on memset data keep the array busy so the
            # first real projections run at the full 2.4 GHz; the first one
            # only needs the small warm_a memset, which rides the otherwise
            # idle Pool engine so PE starts sooner
            warm_a = consts.tile([128, 128], BF16, name="warm_a")
            nc.gpsimd.memset(warm_a, 1.0)

            # warm the ACT exp table while DMAs stream in
            warm_sb = consts.tile([1, 8], F32)
            nc.vector.memset(warm_sb, 0.0)
            nc.scalar.activation(warm_sb, warm_sb, Act.Exp)
            warm_ps0 = pse.tile([128, 512], F32, tag="e", name="wps_first")
            for wj in range(3):
                nc.tensor.matmul(warm_ps0[:, 128 * wj:128 * (wj + 1)],
                                 warm_a, warm_a, start=True, stop=True)
            warm_b = consts.tile([128, 512], BF16, name="warm_b")
            nc.vector.memset(warm_b, 1.0)
            for wi in range(7):
                warm_ps = pse.tile([128, 512], F32, tag="e", name=f"wps{wi}")
                nc.tensor.matmul(warm_ps, warm_a, warm_b, start=True, stop=True)

            # critical-path-first DMA order: the first PE work is the h=0
            # k-projection of columns [0:512), needing wk and x chunk 0
            wT_sb = {}
            t = consts.tile([128, 2, 2, 2, C], F8, name="wk")
            nc.sync.dma_start(out=t, in_=w_d["k"].ap())
            wT_sb["k"] = t
            # x pair tiles: [128, hi/lo, row j, pos] per (half, pair p) —
            # created up front so half 1's stream can prefetch during half 0
            x_sb_all = {}
            for hh in range(NBH):
                x_sb_all[hh] = {}
                for p in range(2):
                    x_sb_all[hh][p] = xpool.tile(
                        [128, 2, 2, KSPAN], F8, tag=f"x{p}", name=f"x{p}_{hh}")

            def _x_dma(hh, p, a, b):
                base = LH * hh
                nc.sync.dma_start(
                    out=x_sb_all[hh][p][:, :, :, a:b],
                    in_=xhl_d.ap()[p][:, :, :, base + a:base + b],
                )

            def _x0_piece(ct, n=1):
                for p in range(2):
                    _x_dma(0, p, 512 * ct, 512 * (ct + n))

            # finer leading pieces: k_group(1)/q_group(0) only need cols
            # [512:1024), so don't make them wait for a 1024-wide transfer
            _x0_piece(0)
            _x0_piece(1)
            t = consts.tile([128, 2, 2, 2, C], F8, name="wq")
            nc.sync.dma_start(out=t, in_=w_d["q"].ap())
            wT_sb["q"] = t
            cb32_sb = consts.tile([128, 264], F32, name="cb32")
            nc.sync.dma_start(out=cb32_sb, in_=cb32_d.ap())
            bq_sb = [cb32_sb[:, 0:1], cb32_sb[:, 1:2]]
            bk_sb = [cb32_sb[:, 2:3], cb32_sb[:, 3:4]]
            padf_sb = [cb32_sb[:, 4:5], cb32_sb[:, 5:6]]
            padl_sb = [cb32_sb[:, 6:7], cb32_sb[:, 7:8]]
            bvrep_sb = cb32_sb[:, 8:264]
            _x0_piece(2)
            t = consts.tile([128, 2, 2, 2, C], F8, name="wv")
            nc.sync.dma_start(out=t, in_=w_d["v"].ap())
            wT_sb["v"] = t
            _x0_piece(3)
            _x0_piece(4)

            cb16_sb = consts.tile([128, 1152], F16, name="cb16")
            nc.sync.dma_start(out=cb16_sb, in_=cb16_d.ap())
            ones_sb = cb16_sb[:, 0:128]
            woT_sb = [cb16_sb[:, 128:640], cb16_sb[:, 640:1152]]
            wo8_sb = consts.tile([128, 2, 2, CIN], F8, name="wo8")
            nc.sync.dma_start(out=wo8_sb, in_=wo8_d.ap())

            opending = []  # tail_a outputs awaiting their deferred outproj
            for h in range(NBH):
                base = LH * h  # halo-coord start of this half's x1/k/v span
                x_sb = x_sb_all[h]

                # ---- projections (fp8 DoubleRow, 3 hi/lo chains) ----
                q_sb, k_sb = [], []
                for cc in range(2):
                    q_sb.append(qkv.tile([128, LH], F16, tag=f"q{cc}", name=f"q{cc}_{h}"))
                    k_sb.append(qkv.tile([128, KSPAN], F16, tag=f"k{cc}", name=f"k{cc}_{h}"))

                CHAINS = ((0, 0), (1, 0), (0, 1))  # (w hi/lo, x hi/lo)

                def _proj_psum(kind, cc, ps, x0):
                    # accumulate W^T x into ps[128, 512] over K=512 via
                    # 2 DoubleRow pair-steps x 3 chains x 2 col-halves
                    csl = slice(128 * cc, 128 * (cc + 1))
                    for half_i in range(2):
                        n0 = x0 + 256 * half_i
                        first = True
                        for p in range(2):
                            for (wp, xp) in CHAINS:
                                nc.tensor.matmul(
                                    ps[:, 256 * half_i:256 * (half_i + 1)],
                                    wT_sb[kind][:, wp, p, :, csl],
                                    x_sb[p][:, xp, :, n0:n0 + 256],
                                    start=first, stop=(p == 1 and (wp, xp) == CHAINS[-1]),
                                    perf_mode=PM, skip_group_check=True,
                                )
                                first = False

                def k_group(mt):
                    for cc in range(2):
                        ps = pse.tile([128, 512], F32, tag="e",
                                      name=f"psk{h}{cc}{mt}")
                        _proj_psum("k", cc, ps, 512 * mt)
                        # k evictions ride ACT (Copy + per-partition bias) so
                        # the DVE queue never gates the next energy matmul
                        nc.scalar.activation(
                            k_sb[cc][:, 512 * mt:512 * (mt + 1)], ps,
                            Act.Identity, bias=bk_sb[cc]
                        )

                def q_group(lt):
                    for cc in range(2):
                        ps = pse.tile([128, 512], F32, tag="e",
                                      name=f"psq{h}{cc}{lt}")
                        _proj_psum("q", cc, ps, HALF + 512 * lt)
                        nc.vector.tensor_scalar_add(
                            q_sb[cc][:, 512 * lt:512 * (lt + 1)], ps, bq_sb[cc]
                        )

                vT_sb = [None] * (KSPAN // 128)
                if h > 0:
                    # halo reuse: this half's m=0..3 v-chunks cover the same
                    # x1 columns as the previous half's m=16..19 — alias them
                    for mt in range(4):
                        vT_sb[mt] = prev_vT[16 + mt]

                def vT_group(mts):
                    for mt in mts:
                        ps = pso.tile([128, C], F32, tag="o", name=f"psv{h}{mt}")
                        first = True
                        for p in range(2):
                            for (wp, xp) in CHAINS:
                                nc.tensor.matmul(
                                    ps,
                                    x_sb[p][:, xp, :, 128 * mt:128 * (mt + 1)],
                                    wT_sb["v"][:, wp, p],
                                    start=first, stop=(p == 1 and (wp, xp) == CHAINS[-1]),
                                    perf_mode=PM, skip_group_check=True,
                                )
                                first = False
                        t = qkv.tile([128, C], F16, tag=f"v{mt}", name=f"vT{mt}_{h}")
                        # eviction with the (per-free-element) v bias folded in
                        nc.vector.tensor_tensor(t, ps, bvrep_sb, op=Alu.add)
                        vT_sb[mt] = t

                # ---- attention blocks (software-pipelined: block b's
                # colsum/AV/outproj are emitted after block b+1's energy+exp
                # so PE never waits on the ACT/Pool softmax chain) ----
                def emit_energy(h, b, k_sb=k_sb, q_sb=q_sb, vT_sb=vT_sb,
                                kh=(kh_prev if h > 0 else None)):
                    woff = 512 * b   # window start in k/vT coords
                    first_blk = (h == 0 and b == 0)
                    last_blk = (h == NBH - 1 and b == BPH - 1)
                    pt = {}
                    for r in AVORDER:
                        lo, w = INTERVALS[r]
                        ps_e = pse.tile([128, w], F32, tag="e", name=f"pse{h}{b}{r}")
                        for cc in range(2):
                            if kh is not None and b == 0 and r < 4:
                                klhs = kh[cc][:, 128 * r:128 * (r + 1)]
                            else:
                                klhs = k_sb[cc][:, woff + 128 * r:
                                                woff + 128 * (r + 1)]
                            nc.tensor.matmul(
                                ps_e, klhs,
                                q_sb[cc][:, 512 * b + lo: 512 * b + lo + w],
                                start=(cc == 0), stop=(cc == 1),
                                skip_group_check=True,
                            )
                        t = ptp.tile([128, w], F16, tag=f"pt{r}", name=f"pt{r}_{h}{b}")
                        nc.scalar.activation(t, ps_e, Act.Exp, scale=EXP_SCALE)
                        # zero outside the band: one affine compare per tile
                        # (lower bound bites for r<=3, upper for r>=4)
                        if r <= 3:
                            nc.gpsimd.affine_select(
                                out=t, in_=t, compare_op=Alu.is_ge, fill=0.0,
                                base=128 * r - lo, channel_multiplier=1,
                                pattern=[[-1, w]],
                            )
                        else:
                            # valid iff (128r+m')-l <= 511, recast as
                            # (511-128r+lo) - m' + j >= 0 (is_ge only)
                            nc.gpsimd.affine_select(
                                out=t, in_=t, compare_op=Alu.is_ge, fill=0.0,
                                base=(BL - 1) - 128 * r + lo,
                                channel_multiplier=-1,
                                pattern=[[1, w]],
                            )
                        if first_blk and r < 2:
                            nc.vector.tensor_scalar_mul(t, t, padf_sb[r])
                        elif last_blk and r >= 6:
                            nc.vector.tensor_scalar_mul(t, t, padl_sb[r - 6])
                        pt[r] = t
                    return (h, b, pt, vT_sb)

                def emit_tail_a(ctx):
                    # colsum -> recip, AV, relu (+ hi/lo fp8 split of relu for
                    # the DoubleRow output projection). The outproj itself is
                    # deferred one more block (emit_tail_b) so its cross-engine
                    # relu chain never stalls PE.
                    h, b, pt, vT_l = ctx
                    ps_s = pss.tile([128, 512], F32, tag="s", name=f"pss{h}{b}")
                    for i, r in enumerate(AVORDER):
                        lo, w = INTERVALS[r]
                        nc.tensor.matmul(
                            ps_s[:, lo:lo + w], ones_sb, pt[r],
                            start=(i == 0), stop=(i == 7), skip_group_check=True,
                        )
                    recip = sbo.tile([128, 512], F32, tag="recip", name=f"rc{h}{b}")
                    nc.vector.reciprocal(recip, ps_s)
                    ps_av = []
                    for cc in range(2):
                        ps_av.append(psav.tile([128, 512], F32, tag=f"av{cc}",
                                               name=f"psav{h}{b}{cc}"))
                    for i, r in enumerate(AVORDER):
                        lo, w = INTERVALS[r]
                        for cc in range(2):
                            nc.tensor.matmul(
                                ps_av[cc][:, lo:lo + w],
                                vT_l[4 * b + r][:, 128 * cc:128 * (cc + 1)], pt[r],
                                start=(i == 0), stop=(i == 7), skip_group_check=True,
                            )
                    relu_sb = []
                    for cc in range(2):
                        relu_sb.append(sbo.tile([128, 512], F16, tag=f"relu{cc}",
                                                name=f"relu{h}{b}{cc}"))
                    # normalization fused into the relu eviction: keeps the
                    # fp16 tile in range and shortens the output tail
                    for cc in range(2):
                        nc.vector.scalar_tensor_tensor(
                            relu_sb[cc], ps_av[cc], 0.0, recip,
                            op0=Alu.max, op1=Alu.mult,
                        )
                    use_fp8 = not (h == NBH - 1 and b == BPH - 2)
                    r8h = r8l = None
                    if use_fp8:
                        r8h = sbo.tile([128, 2, 512], F8, tag="r8h",
                                       name=f"r8h{h}{b}")
                        r8l = sbo.tile([128, 2, 512], F8, tag="r8l",
                                       name=f"r8l{h}{b}")
                        for cc in range(2):
                            nc.scalar.activation(r8h[:, cc], relu_sb[cc],
                                                 Act.Copy)
                            # SBUF-only subtract -> legal on Pool/GPSIMD; the
                            # one-block tail_b deferral gives it ample slack
                            nc.gpsimd.tensor_tensor(
                                r8l[:, cc], relu_sb[cc], r8h[:, cc],
                                op=Alu.subtract)
                    return (h, b, relu_sb, r8h, r8l, use_fp8)

                def emit_tail_b(octx):
                    h, b, relu_sb, r8h, r8l, use_fp8 = octx
                    c0 = LH * h + 512 * b
                    o_sb = sbo.tile([128, 4, 512], F16, tag="osb", name=f"o{h}{b}")
                    for oc in range(4):
                        ps_o = pso.tile([128, 512], F32, tag="o",
                                        name=f"pso{h}{b}{oc}")
                        if use_fp8:
                            for pc in range(2):
                                psl = slice(256 * pc, 256 * (pc + 1))
                                for ci, (wp, rp) in enumerate(
                                        ((0, r8h), (1, r8h), (0, r8l))):
                                    nc.tensor.matmul(
                                        ps_o[:, psl],
                                        wo8_sb[:, wp, :, 128 * oc:128 * (oc + 1)],
                                        rp[:, :, psl],
                                        start=(ci == 0), stop=(ci == 2),
                                        perf_mode=PM, skip_group_check=True,
                                    )
                            nc.scalar.activation(o_sb[:, oc], ps_o, Act.Copy,
                                                 scale=1.0 / 1024.0)
                        else:
                            for cc in range(2):
                                nc.tensor.matmul(
                                    ps_o, woT_sb[cc][:, 128 * oc:128 * (oc + 1)],
                                    relu_sb[cc], start=(cc == 0), stop=(cc == 1),
                                )
                            nc.scalar.activation(o_sb[:, oc], ps_o, Act.Copy)
                        if oc % 2:
                            nc.sync.dma_start(
                                out=out_d.ap()[:, oc - 1:oc + 1, c0:c0 + 512],
                                in_=o_sb[:, oc - 1:oc + 1],
                            )

                def emit_tail_last(ctx):
                    # final block, fused fp16 path: AV order ending with the
                    # l>=256 chunks so the left output half drains while AV
                    # finishes — the pipeline is empty after this block and
                    # every exposed serial step is pure tail latency
                    h, b, pt, vT_l = ctx
                    ps_s = pss.tile([128, 512], F32, tag="s", name=f"pss{h}{b}")
                    for i, r in enumerate(AVORDER):
                        lo, w = INTERVALS[r]
                        nc.tensor.matmul(
                            ps_s[:, lo:lo + w], ones_sb, pt[r],
                            start=(i == 0), stop=(i == 7), skip_group_check=True,
                        )
                    recip = sbo.tile([128, 512], F32, tag="recip", name=f"rc{h}{b}")
                    nc.vector.reciprocal(recip, ps_s)
                    avorder = [3, 4, 2, 5, 1, 0, 6, 7]
                    ps_av = []
                    for cc in range(2):
                        ps_av.append(psav.tile([128, 512], F32, tag=f"av{cc}",
                                               name=f"psav{h}{b}{cc}"))
                    relu_sb = []
                    for cc in range(2):
                        relu_sb.append(sbo.tile([128, 512], F16, tag=f"relu{cc}",
                                                name=f"relu{h}{b}{cc}"))
                    o_sb = sbo.tile([128, 4, 512], F16, tag="osb", name=f"o{h}{b}")
                    done = 0
                    for (l0, l1), steps in (((0, 256), 6), ((256, 512), 8)):
                        for i in range(done, steps):
                            r = avorder[i]
                            lo, w = INTERVALS[r]
                            for cc in range(2):
                                nc.tensor.matmul(
                                    ps_av[cc][:, lo:lo + w],
                                    vT_l[4 * b + r][:, 128 * cc:128 * (cc + 1)], pt[r],
                                    start=(i == 0), stop=(i == steps - 1),
                                    skip_group_check=True,
                                )
                        done = steps
                        lsl = slice(l0, l1)
                        c0 = LH * h + 512 * b
                        for cc in range(2):
                            nc.vector.scalar_tensor_tensor(
                                relu_sb[cc][:, lsl], ps_av[cc][:, lsl], 0.0,
                                recip[:, lsl], op0=Alu.max, op1=Alu.mult,
                            )
                        for oc in range(4):
                            ps_o = pso.tile([128, 512], F32, tag="o",
                                            name=f"pso{h}{b}{oc}{l0}")
                            for cc in range(2):
                                nc.tensor.matmul(
                                    ps_o[:, lsl],
                                    woT_sb[cc][:, 128 * oc:128 * (oc + 1)],
                                    relu_sb[cc][:, lsl],
                                    start=(cc == 0), stop=(cc == 1),
                                )
                            # alternate ACT/DVE so the final evictions don't
                            # serialize on one engine
                            if oc % 2:
                                nc.vector.scalar_tensor_tensor(
                                    o_sb[:, oc, lsl], ps_o[:, lsl], 0.0,
                                    recip[:, lsl], op0=Alu.bypass, op1=Alu.bypass,
                                )
                            else:
                                nc.scalar.activation(o_sb[:, oc, lsl], ps_o[:, lsl],
                                                     Act.Copy)
                            # oc-pair copies at half granularity: early issue
                            # overlaps the remaining evictions without filling
                            # the drain path with per-quarter issue overhead
                            if oc % 2 and l1 % 256 == 0:
                                nc.sync.dma_start(
                                    out=out_d.ap()[:, oc - 1:oc + 1,
                                                   c0 + l1 - 256: c0 + l1],
                                    in_=o_sb[:, oc - 1:oc + 1, l1 - 256:l1],
                                )

                pending = []
                for b in range(BPH):
                    if b == 0:
                        if h == 0:
                            k_group(0)
                        k_group(1)
                        q_group(0)
                        vT_group(range(0, 8) if h == 0 else range(4, 8))
                    else:
                        k_group(b + 1)
                        q_group(b)
                        vT_group(range(4 * b + 4, 4 * b + 8))
                    # prefetch the next half's x stream while this half still
                    # computes — its tiles are fresh buffers (xpool bufs=2)
                    if h + 1 < NBH and b == 2:
                        for p in range(2):
                            _x_dma(h + 1, p, 256, 1536)
                    elif h + 1 < NBH and b == 3:
                        for p in range(2):
                            _x_dma(h + 1, p, 1536, 2560)
                    pending.append(emit_energy(h, b))
                    if len(pending) > 1:
                        opending.append(emit_tail_a(pending.pop(0)))
                    if len(opending) > 1:
                        emit_tail_b(opending.pop(0))
                # the AV/relu of this half's final block must flush before the
                # next half's projections overwrite vT; its outproj (and any
                # older pending outproj) needs only sbo tiles + weights, so it
                # slides into the next half's stream — except at the very end,
                # where the final block takes the fused low-latency path
                if h < NBH - 1:
                    opending.append(emit_tail_a(pending.pop(0)))
                else:
                    while opending:
                        emit_tail_b(opending.pop(0))
                    emit_tail_last(pending.pop(0))
                prev_vT = vT_sb
                if h == 0:
                    # stash the k halo overlap for the next half (SBUF->SBUF
                    # DMA, off-engine); half1's block 0 reads it directly
                    kh_prev = []
                    for cc in range(2):
                        tkh = qkv.tile([128, 512], F16, tag=f"kh{cc}",
                                       name=f"kh{cc}")
                        nc.sync.dma_start(out=tkh,
                                          in_=k_sb[cc][:, LH:LH + 512])
                        kh_prev.append(tkh)
    nc.compile()
    return nc


_NC_CACHE = {}


def _get_nc():
    if "nc" not in _NC_CACHE:
        _NC_CACHE["nc"] = _build_program()
    return _NC_CACHE["nc"]


def _f8():
    try:
        import ml_dtypes
        return ml_dtypes.float8_e4m3
    except ImportError:  # pragma: no cover
        import jax.numpy as jnp
        return jnp.float8_e4m3


def _split8(a):
    f8 = _f8()
    hi = np.asarray(a, np.float32).astype(f8)
    lo = (np.asarray(a, np.float32) - hi.astype(np.float32)).astype(f8)
    return hi, lo


def make_in_maps(x1, mask, Wq, bq, Wk, bk, Wv, bv, Wo, bo):
    x1 = np.asarray(x1, dtype=np.float32).reshape(CIN, L)

    def _pairs(w):
        # (C_out=256, C_in=512) -> [128, 2(hl), 2(p), 2(j), C] DoubleRow pair
        # layout with global c_in = 128*(2p+j) + c_in_within
        ws = np.asarray(w, np.float32) * WSCALE
        hi, lo = _split8(ws.T)          # (512, 256) each
        def lay(a):
            return a.reshape(2, 2, 128, C).transpose(2, 0, 1, 3)
        return np.ascontiguousarray(np.stack([lay(hi), lay(lo)], axis=1))

    wq8 = _pairs(Wq)
    wk8 = _pairs(Wk)
    wv8 = _pairs(Wv)
    woT = (np.asarray(Wo, np.float32).T / WSCALE).astype(np.float16)
    # fp8 hi/lo of 16*Wo^T in [c_within, hl, cc, c_out] layout (the 16*64
    # product scaling is undone by the 2^-10 eviction scale)
    wo_hi, wo_lo = _split8(np.asarray(Wo, np.float32).T * 16.0)
    def _wo_lay(a):
        return a.reshape(2, 128, CIN).transpose(1, 0, 2)
    wo8 = np.ascontiguousarray(
        np.stack([_wo_lay(wo_hi), _wo_lay(wo_lo)], axis=1))

    cb32 = np.zeros((128, 264), np.float32)
    cb32[:, 0:2] = (np.asarray(bq, np.float32) * WSCALE).reshape(2, 128).T
    cb32[:, 2:4] = (np.asarray(bk, np.float32) * WSCALE).reshape(2, 128).T
    # padf/padl cols 4:8 filled per core below
    cb32[:, 8:264] = np.broadcast_to(
        (np.asarray(bv, np.float32) * WSCALE).reshape(1, C), (128, C))

    cb16 = np.zeros((128, 1152), np.float16)
    cb16[:, 0:128] = 1.0
    cb16[:, 128:640] = woT[0:128]
    cb16[:, 640:1152] = woT[128:256]

    in_maps = []
    for c in range(NCORES):
        g0 = LC * c - HALF
        x1h = np.zeros((CIN, HALO), np.float32)
        s0, s1 = max(g0, 0), min(g0 + HALO, L)
        x1h[:, s0 - g0:s1 - g0] = x1[:, s0:s1]
        xh, xl = _split8(x1h)
        # [p, c_within, hl, j, pos] with global channel 128*(2p+j) + c_within
        xhl = np.stack([xh.reshape(2, 2, 128, HALO),
                        xl.reshape(2, 2, 128, HALO)], axis=0)
        xhl = np.ascontiguousarray(xhl.transpose(1, 3, 0, 2, 4))
        cb = cb32.copy()
        cb[:, 4:6] = 0.0 if c == 0 else 1.0
        cb[:, 6:8] = 0.0 if c == NCORES - 1 else 1.0
        m = {
            "xhl": xhl,
            "wq": wq8, "wk": wk8, "wv": wv8, "wo8": wo8,
            "cb32": cb, "cb16": cb16,
        }
        in_maps.append(m)
    return in_maps


def postprocess(results, mask, bo):
    # per-core out is [128, 4, LC] c-major; channel g = 128*oc + c
    cols = np.concatenate(
        [np.asarray(results[c]["out"], np.float32).transpose(1, 0, 2)
         .reshape(CIN, LC) for c in range(NCORES)], axis=1)
    out = cols[None] + np.asarray(bo, np.float32)[None, :, None]
    return (out * np.asarray(mask, np.float32)).astype(np.float32)


def kernel(x1, x2, mask, Wq, bq, Wk, bk, Wv, bv, Wo, bo, **_unused):
    from concourse.bass_utils import run_bass_kernel_spmd

    nc = _get_nc()
    in_maps = make_in_maps(x1, mask, Wq, bq, Wk, bk, Wv, bv, Wo, bo)
    res = run_bass_kernel_spmd(nc, in_maps, core_ids=list(range(NCORES)))
    return postprocess(res.results, mask, bo)


# revision 54
# speedup vs baseline: 1.0075x; 1.0075x over previous
"""Sliding-window (banded) attention for nn_AttLayer on 8 Trainium2 NeuronCores.

Reference computation (per window-block n of 512 positions, 64 blocks over L=32768):
  q/k/v = 1x1-conv projections of x1 (512ch -> 256ch)
  energy[l, m] = (q_block[:, l] . k_window[:, m]) / 16   over a 1024-wide window
  attn = softmax(energy + log(band_mask + 1e-6)) * band_mask
  out  = relu(v_window @ attn^T) -> 1x1-conv (256 -> 512) + bias, masked

Sharding: 64 blocks split contiguously across 8 cores (8 blocks each). Each core
gets a zero-padded halo slice of x1 and computes its 4096 output columns.

Kernel strategy (per core, SPMD — all per-core variation is in the data):
  - Projections on PE in fp8e4 DoubleRow perf mode (0.5 cycles/row, two
    128-channel contraction tiles per instruction -> 4x MAC throughput).
    x1 and the projection weights are split host-side into e4m3 hi/lo pairs
    (W scaled by 64 to center the fp8 range; the 64^2 folds into the exp
    scale and 1/64 into Wo). Three accumulation chains (hi*hi + lo*hi +
    hi*lo) recover ~bf16-level accuracy at 0.75x the f32r cycle cost.
  - Everything downstream runs in fp16 (1 cycle/row like f32r but with no
    >=256 moving-width requirement, half the SBUF/DMA bytes of f32, and a
    10-bit mantissa -- ~10x less quantization error than bf16).
  - energy computed transposed: energyT[m, l] = k_chunk^T q (PE), over the
    EXACT per-chunk band intervals (2560 of 4096 window cols per block).
  - Band masking: affine_select on the otherwise-idle Pool/GPSIMD engine
    zeroes out-of-band exp values; sequence-edge padding handled with
    per-core 0/1 data vectors so the program stays SPMD.
  - exp on ScalarE (scale 2^-16 folds away the fp8 weight scaling),
    denominators via an all-ones fp16 matmul (column sums land replicated
    across partitions), reciprocal on VectorE.
  - AV on PE (fp16); softmax normalization + relu fused into the one
    PSUM->SBUF eviction (scalar_tensor_tensor max+mult) which also keeps
    the fp16 relu tile in range. Output projection in fp8 DoubleRow too
    (host-split Wo hi/lo x relu hi/lo, the relu split riding ACT+Pool
    slack); k evictions ride ACT (Identity + bias) so the DVE queue never
    gates energy. Final bias/mask applied on host.
  - Two-deep software pipeline: block b's colsum/AV/relu (tail_a) are
    emitted after block b+1's energy, and its output projection (tail_b)
    one block later still, so neither the softmax chain nor the relu
    hi/lo split chain ever stalls PE. The tail_b queue carries across the
    half boundary, covering the h0->h1 transition; half 1's x stream is
    prefetched during half 0 (xpool bufs=2).
  - Halo reuse between the two halves: half 1's first four vT tiles alias
    half 0's last four (identical x1 columns), and half 0's k overlap is
    stashed via an SBUF->SBUF DMA so half 1 skips its first k-group.
  - The final block takes a fused low-latency fp16 path (AV l-halves with
    reordered accumulation, ACT/DVE-alternating evictions, oc-pair output
    DMAs) because everything it exposes is pure drain-path latency.
"""

import numpy as np

NCORES = 8
L = 32768
CIN = 512
C = 256
BL = 512
HALF = 256
LC = L // NCORES              # 4096 positions per core
HALO = LC + 2 * HALF          # 4608
NBH = 2                       # halves per core
LH = LC // NBH                # 2048 positions per half
KSPAN = LH + 2 * HALF         # 2560 k/v positions per half
BPH = 4                       # blocks per half
WSCALE = 64.0                 # host-side fp8 scaling of Wq/Wk/Wv (and biases)
EXP_SCALE = (1.0 / 16.0) / (WSCALE * WSCALE)   # softmax scale / W-scaling^2

# Per m-chunk r (8 chunks of the 1024-wide window): EXACT valid l-interval
# (lo, width) within the block's 512 queries (fp16 has no min-width penalty).
INTERVALS = [
    (0, 128), (0, 256), (0, 384), (0, 512),
    (0, 512), (128, 384), (256, 256), (384, 128),
]
# accumulation order: r=3 covers the full [0,512) so it goes first (start=True)
AVORDER = [3, 4, 2, 5, 1, 6, 0, 7]


def _build_program():
    import concourse.mybir as mybir
    from concourse import bacc
    from concourse.tile import TileContext

    F32 = mybir.dt.float32
    F16 = mybir.dt.float16
    F8 = mybir.dt.float8e4
    BF16 = mybir.dt.bfloat16
    Alu = mybir.AluOpType
    Act = mybir.ActivationFunctionType
    PM = mybir.MatmulPerfMode.DoubleRow

    nc = bacc.Bacc()

    # x hi/lo fp8 halo slice in SBUF-tile order [pair p, c_within(128), hi/lo,
    # row j, pos] so ONE DMA per pair fills hi+lo and both j rows; weights in
    # DoubleRow pair layout [c_in_within_chunk(128), hi/lo, pair p, row j,
    # c_out] with global input channel 128*(2p+j) + c_in.
    xhl_d = nc.dram_tensor("xhl", [2, 128, 2, 2, HALO], F8, kind="ExternalInput")
    w_d = {}
    for kind in ("q", "k", "v"):
        w_d[kind] = nc.dram_tensor(
            f"w{kind}", [128, 2, 2, 2, C], F8, kind="ExternalInput")
    # f32 scalar blob: [bq0 bq1 bk0 bk1 padf0 padf1 padl0 padl1 | bvr(256)]
    cb32_d = nc.dram_tensor("cb32", [128, 264], F32, kind="ExternalInput")
    # f16 blob: [ones(128) | woT0(512) | woT1(512)]
    cb16_d = nc.dram_tensor("cb16", [128, 1152], F16, kind="ExternalInput")
    # fp8 hi/lo output-projection weights [c_in_within, hi/lo, cc, c_out]
    wo8_d = nc.dram_tensor("wo8", [128, 2, 2, CIN], F8, kind="ExternalInput")
    # c-major output: full channel g = 128*oc + c lives at [c, oc, :] — lets
    # one DMA carry several oc chunks (host re-interleaves)
    out_d = nc.dram_tensor("out", [128, 4, LC], F16, kind="ExternalOutput")

    with TileContext(nc) as tc:
        with (
            tc.tile_pool(name="consts", bufs=1) as consts,
            tc.tile_pool(name="xpool", bufs=2) as xpool,
            tc.tile_pool(name="qkv", bufs=1) as qkv,
            tc.tile_pool(name="ptp", bufs=2) as ptp,
            tc.tile_pool(name="sbo", bufs=4) as sbo,
            tc.tile_pool(name="pse", bufs=3, space="PSUM") as pse,
            tc.tile_pool(name="pss", bufs=1, space="PSUM") as pss,
            tc.tile_pool(name="psav", bufs=1, space="PSUM") as psav,
            tc.tile_pool(name="pso", bufs=2, space="PSUM") as pso,
        ):
            # warm the PE clock gate (HAM) during the initial DMA wait:
            # dummy bf16 matmuls on memset data keep the array busy so the
            # first real projections run at the full 2.4 GHz; the first one
            # only needs the small warm_a memset, which rides the otherwise
            # idle Pool engine so PE starts sooner
            warm_a = consts.tile([128, 128], BF16, name="warm_a")
            nc.gpsimd.memset(warm_a, 1.0)

            # warm the ACT exp table while DMAs stream in
            warm_sb = consts.tile([1, 8], F32)
            nc.vector.memset(warm_sb, 0.0)
            nc.scalar.activation(warm_sb, warm_sb, Act.Exp)
            warm_ps0 = pse.tile([128, 512], F32, tag="e", name="wps_first")
            for wj in range(3):
                nc.tensor.matmul(warm_ps0[:, 128 * wj:128 * (wj + 1)],
                                 warm_a, warm_a, start=True, stop=True)
            warm_b = consts.tile([128, 512], BF16, name="warm_b")
            nc.vector.memset(warm_b, 1.0)
            for wi in range(7):
                warm_ps = pse.tile([128, 512], F32, tag="e", name=f"wps{wi}")
                nc.tensor.matmul(warm_ps, warm_a, warm_b, start=True, stop=True)

            # critical-path-first DMA order: the first PE work is the h=0
            # k-projection of columns [0:512), needing wk and x chunk 0
            wT_sb = {}
            t = consts.tile([128, 2, 2, 2, C], F8, name="wk")
            nc.sync.dma_start(out=t, in_=w_d["k"].ap())
            wT_sb["k"] = t
            # x pair tiles: [128, hi/lo, row j, pos] per (half, pair p) —
            # created up front so half 1's stream can prefetch during half 0
            x_sb_all = {}
            for hh in range(NBH):
                x_sb_all[hh] = {}
                for p in range(2):
                    x_sb_all[hh][p] = xpool.tile(
                        [128, 2, 2, KSPAN], F8, tag=f"x{p}", name=f"x{p}_{hh}")

            def _x_dma(hh, p, a, b):
                base = LH * hh
                nc.sync.dma_start(
                    out=x_sb_all[hh][p][:, :, :, a:b],
                    in_=xhl_d.ap()[p][:, :, :, base + a:base + b],
                )

            def _x0_piece(ct, n=1):
                for p in range(2):
                    _x_dma(0, p, 512 * ct, 512 * (ct + n))

            # finer leading pieces: k_group(1)/q_group(0) only need cols
            # [512:1024), so don't make them wait for a 1024-wide transfer
            _x0_piece(0)
            cb32_sb = consts.tile([128, 264], F32, name="cb32")
            nc.sync.dma_start(out=cb32_sb, in_=cb32_d.ap())
            _x0_piece(1)
            t = consts.tile([128, 2, 2, 2, C], F8, name="wq")
            nc.sync.dma_start(out=t, in_=w_d["q"].ap())
            wT_sb["q"] = t
            bq_sb = [cb32_sb[:, 0:1], cb32_sb[:, 1:2]]
            bk_sb = [cb32_sb[:, 2:3], cb32_sb[:, 3:4]]
            padf_sb = [cb32_sb[:, 4:5], cb32_sb[:, 5:6]]
            padl_sb = [cb32_sb[:, 6:7], cb32_sb[:, 7:8]]
            bvrep_sb = cb32_sb[:, 8:264]
            _x0_piece(2)
            t = consts.tile([128, 2, 2, 2, C], F8, name="wv")
            nc.sync.dma_start(out=t, in_=w_d["v"].ap())
            wT_sb["v"] = t
            _x0_piece(3)
            _x0_piece(4)

            cb16_sb = consts.tile([128, 1152], F16, name="cb16")
            nc.sync.dma_start(out=cb16_sb, in_=cb16_d.ap())
            ones_sb = cb16_sb[:, 0:128]
            woT_sb = [cb16_sb[:, 128:640], cb16_sb[:, 640:1152]]
            wo8_sb = consts.tile([128, 2, 2, CIN], F8, name="wo8")
            nc.sync.dma_start(out=wo8_sb, in_=wo8_d.ap())

            opending = []  # tail_a outputs awaiting their deferred outproj
            for h in range(NBH):
                base = LH * h  # halo-coord start of this half's x1/k/v span
                x_sb = x_sb_all[h]

                # ---- projections (fp8 DoubleRow, 3 hi/lo chains) ----
                q_sb, k_sb = [], []
                for cc in range(2):
                    q_sb.append(qkv.tile([128, LH], F16, tag=f"q{cc}", name=f"q{cc}_{h}"))
                    k_sb.append(qkv.tile([128, KSPAN], F16, tag=f"k{cc}", name=f"k{cc}_{h}"))

                CHAINS = ((0, 0), (1, 0), (0, 1))  # (w hi/lo, x hi/lo)

                def _proj_psum(kind, cc, ps, x0):
                    # accumulate W^T x into ps[128, 512] over K=512 via
                    # 2 DoubleRow pair-steps x 3 chains x 2 col-halves
                    csl = slice(128 * cc, 128 * (cc + 1))
                    for half_i in range(2):
                        n0 = x0 + 256 * half_i
                        first = True
                        for p in range(2):
                            for (wp, xp) in CHAINS:
                                nc.tensor.matmul(
                                    ps[:, 256 * half_i:256 * (half_i + 1)],
                                    wT_sb[kind][:, wp, p, :, csl],
                                    x_sb[p][:, xp, :, n0:n0 + 256],
                                    start=first, stop=(p == 1 and (wp, xp) == CHAINS[-1]),
                                    perf_mode=PM, skip_group_check=True,
                                )
                                first = False

                def k_group(mt):
                    for cc in range(2):
                        ps = pse.tile([128, 512], F32, tag="e",
                                      name=f"psk{h}{cc}{mt}")
                        _proj_psum("k", cc, ps, 512 * mt)
                        # k evictions ride ACT (Copy + per-partition bias) so
                        # the DVE queue never gates the next energy matmul
                        nc.scalar.activation(
                            k_sb[cc][:, 512 * mt:512 * (mt + 1)], ps,
                            Act.Identity, bias=bk_sb[cc]
                        )

                def q_group(lt):
                    for cc in range(2):
                        ps = pse.tile([128, 512], F32, tag="e",
                                      name=f"psq{h}{cc}{lt}")
                        _proj_psum("q", cc, ps, HALF + 512 * lt)
                        nc.vector.tensor_scalar_add(
                            q_sb[cc][:, 512 * lt:512 * (lt + 1)], ps, bq_sb[cc]
                        )

                vT_sb = [None] * (KSPAN // 128)
                if h > 0:
                    # halo reuse: this half's m=0..3 v-chunks cover the same
                    # x1 columns as the previous half's m=16..19 — alias them
                    for mt in range(4):
                        vT_sb[mt] = prev_vT[16 + mt]

                def vT_group(mts):
                    for mt in mts:
                        ps = pso.tile([128, C], F32, tag="o", name=f"psv{h}{mt}")
                        first = True
                        for p in range(2):
                            for (wp, xp) in CHAINS:
                                nc.tensor.matmul(
                                    ps,
                                    x_sb[p][:, xp, :, 128 * mt:128 * (mt + 1)],
                                    wT_sb["v"][:, wp, p],
                                    start=first, stop=(p == 1 and (wp, xp) == CHAINS[-1]),
                                    perf_mode=PM, skip_group_check=True,
                                )
                                first = False
                        t = qkv.tile([128, C], F16, tag=f"v{mt}", name=f"vT{mt}_{h}")
                        # eviction with the (per-free-element) v bias folded in
                        nc.vector.tensor_tensor(t, ps, bvrep_sb, op=Alu.add)
                        vT_sb[mt] = t

                # ---- attention blocks (software-pipelined: block b's
                # colsum/AV/outproj are emitted after block b+1's energy+exp
                # so PE never waits on the ACT/Pool softmax chain) ----
                def emit_energy(h, b, k_sb=k_sb, q_sb=q_sb, vT_sb=vT_sb,
                                kh=(kh_prev if h > 0 else None)):
                    woff = 512 * b   # window start in k/vT coords
                    first_blk = (h == 0 and b == 0)
                    last_blk = (h == NBH - 1 and b == BPH - 1)
                    pt = {}
                    for r in AVORDER:
                        lo, w = INTERVALS[r]
                        ps_e = pse.tile([128, w], F32, tag="e", name=f"pse{h}{b}{r}")
                        for cc in range(2):
                            if kh is not None and b == 0 and r < 4:
                                klhs = kh[cc][:, 128 * r:128 * (r + 1)]
                            else:
                                klhs = k_sb[cc][:, woff + 128 * r:
                                                woff + 128 * (r + 1)]
                            nc.tensor.matmul(
                                ps_e, klhs,
                                q_sb[cc][:, 512 * b + lo: 512 * b + lo + w],
                                start=(cc == 0), stop=(cc == 1),
                                skip_group_check=True,
                            )
                        t = ptp.tile([128, w], F16, tag=f"pt{r}", name=f"pt{r}_{h}{b}")
                        nc.scalar.activation(t, ps_e, Act.Exp, scale=EXP_SCALE)
                        # zero outside the band: one affine compare per tile
                        # (lower bound bites for r<=3, upper for r>=4)
                        if r <= 3:
                            nc.gpsimd.affine_select(
                                out=t, in_=t, compare_op=Alu.is_ge, fill=0.0,
                                base=128 * r - lo, channel_multiplier=1,
                                pattern=[[-1, w]],
                            )
                        else:
                            # valid iff (128r+m')-l <= 511, recast as
                            # (511-128r+lo) - m' + j >= 0 (is_ge only)
                            nc.gpsimd.affine_select(
                                out=t, in_=t, compare_op=Alu.is_ge, fill=0.0,
                                base=(BL - 1) - 128 * r + lo,
                                channel_multiplier=-1,
                                pattern=[[1, w]],
                            )
                        if first_blk and r < 2:
                            nc.vector.tensor_scalar_mul(t, t, padf_sb[r])
                        elif last_blk and r >= 6:
                            nc.vector.tensor_scalar_mul(t, t, padl_sb[r - 6])
                        pt[r] = t
                    return (h, b, pt, vT_sb)

                def emit_tail_a(ctx):
                    # colsum -> recip, AV, relu (+ hi/lo fp8 split of relu for
                    # the DoubleRow output projection). The outproj itself is
                    # deferred one more block (emit_tail_b) so its cross-engine
                    # relu chain never stalls PE.
                    h, b, pt, vT_l = ctx
                    ps_s = pss.tile([128, 512], F32, tag="s", name=f"pss{h}{b}")
                    for i, r in enumerate(AVORDER):
                        lo, w = INTERVALS[r]
                        nc.tensor.matmul(
                            ps_s[:, lo:lo + w], ones_sb, pt[r],
                            start=(i == 0), stop=(i == 7), skip_group_check=True,
                        )
                    recip = sbo.tile([128, 512], F32, tag="recip", name=f"rc{h}{b}")
                    nc.vector.reciprocal(recip, ps_s)
                    ps_av = []
                    for cc in range(2):
                        ps_av.append(psav.tile([128, 512], F32, tag=f"av{cc}",
                                               name=f"psav{h}{b}{cc}"))
                    for i, r in enumerate(AVORDER):
                        lo, w = INTERVALS[r]
                        for cc in range(2):
                            nc.tensor.matmul(
                                ps_av[cc][:, lo:lo + w],
                                vT_l[4 * b + r][:, 128 * cc:128 * (cc + 1)], pt[r],
                                start=(i == 0), stop=(i == 7), skip_group_check=True,
                            )
                    relu_sb = []
                    for cc in range(2):
                        relu_sb.append(sbo.tile([128, 512], F16, tag=f"relu{cc}",
                                                name=f"relu{h}{b}{cc}"))
                    # normalization fused into the relu eviction: keeps the
                    # fp16 tile in range and shortens the output tail
                    for cc in range(2):
                        nc.vector.scalar_tensor_tensor(
                            relu_sb[cc], ps_av[cc], 0.0, recip,
                            op0=Alu.max, op1=Alu.mult,
                        )
                    use_fp8 = not (h == NBH - 1 and b == BPH - 2)
                    r8h = r8l = None
                    if use_fp8:
                        r8h = sbo.tile([128, 2, 512], F8, tag="r8h",
                                       name=f"r8h{h}{b}")
                        r8l = sbo.tile([128, 2, 512], F8, tag="r8l",
                                       name=f"r8l{h}{b}")
                        for cc in range(2):
                            nc.scalar.activation(r8h[:, cc], relu_sb[cc],
                                                 Act.Copy)
                            # SBUF-only subtract -> legal on Pool/GPSIMD; the
                            # one-block tail_b deferral gives it ample slack
                            nc.gpsimd.tensor_tensor(
                                r8l[:, cc], relu_sb[cc], r8h[:, cc],
                                op=Alu.subtract)
                    return (h, b, relu_sb, r8h, r8l, use_fp8)

                def emit_tail_b(octx):
                    h, b, relu_sb, r8h, r8l, use_fp8 = octx
                    c0 = LH * h + 512 * b
                    o_sb = sbo.tile([128, 4, 512], F16, tag="osb", name=f"o{h}{b}")
                    for oc in range(4):
                        ps_o = pso.tile([128, 512], F32, tag="o",
                                        name=f"pso{h}{b}{oc}")
                        if use_fp8:
                            for pc in range(2):
                                psl = slice(256 * pc, 256 * (pc + 1))
                                for ci, (wp, rp) in enumerate(
                                        ((0, r8h), (1, r8h), (0, r8l))):
                                    nc.tensor.matmul(
                                        ps_o[:, psl],
                                        wo8_sb[:, wp, :, 128 * oc:128 * (oc + 1)],
                                        rp[:, :, psl],
                                        start=(ci == 0), stop=(ci == 2),
                                        perf_mode=PM, skip_group_check=True,
                                    )
                            nc.scalar.activation(o_sb[:, oc], ps_o, Act.Copy,
                                                 scale=1.0 / 1024.0)
                        else:
                            for cc in range(2):
                                nc.tensor.matmul(
                                    ps_o, woT_sb[cc][:, 128 * oc:128 * (oc + 1)],
                                    relu_sb[cc], start=(cc == 0), stop=(cc == 1),
                                )
                            nc.scalar.activation(o_sb[:, oc], ps_o, Act.Copy)
                        if oc % 2:
                            nc.sync.dma_start(
                                out=out_d.ap()[:, oc - 1:oc + 1, c0:c0 + 512],
                                in_=o_sb[:, oc - 1:oc + 1],
                            )

                def emit_tail_last(ctx):
                    # final block, fused fp16 path: AV order ending with the
                    # l>=256 chunks so the left output half drains while AV
                    # finishes — the pipeline is empty after this block and
                    # every exposed serial step is pure tail latency
                    h, b, pt, vT_l = ctx
                    ps_s = pss.tile([128, 512], F32, tag="s", name=f"pss{h}{b}")
                    for i, r in enumerate(AVORDER):
                        lo, w = INTERVALS[r]
                        nc.tensor.matmul(
                            ps_s[:, lo:lo + w], ones_sb, pt[r],
                            start=(i == 0), stop=(i == 7), skip_group_check=True,
                        )
                    recip = sbo.tile([128, 512], F32, tag="recip", name=f"rc{h}{b}")
                    nc.vector.reciprocal(recip, ps_s)
                    avorder = [3, 4, 2, 5, 1, 0, 6, 7]
                    ps_av = []
                    for cc in range(2):
                        ps_av.append(psav.tile([128, 512], F32, tag=f"av{cc}",
                                               name=f"psav{h}{b}{cc}"))
                    relu_sb = []
                    for cc in range(2):
                        relu_sb.append(sbo.tile([128, 512], F16, tag=f"relu{cc}",
                                                name=f"relu{h}{b}{cc}"))
                    o_sb = sbo.tile([128, 4, 512], F16, tag="osb", name=f"o{h}{b}")
                    done = 0
                    for (l0, l1), steps in (((0, 256), 6), ((256, 512), 8)):
                        for i in range(done, steps):
                            r = avorder[i]
                            lo, w = INTERVALS[r]
                            for cc in range(2):
                                nc.tensor.matmul(
                                    ps_av[cc][:, lo:lo + w],
                                    vT_l[4 * b + r][:, 128 * cc:128 * (cc + 1)], pt[r],
                                    start=(i == 0), stop=(i == steps - 1),
                                    skip_group_check=True,
                                )
                        done = steps
                        lsl = slice(l0, l1)
                        c0 = LH * h + 512 * b
                        for cc in range(2):
                            nc.vector.scalar_tensor_tensor(
                                relu_sb[cc][:, lsl], ps_av[cc][:, lsl], 0.0,
                                recip[:, lsl], op0=Alu.max, op1=Alu.mult,
                            )
                        for oc in range(4):
                            ps_o = pso.tile([128, 512], F32, tag="o",
                                            name=f"pso{h}{b}{oc}{l0}")
                            for cc in range(2):
                                nc.tensor.matmul(
                                    ps_o[:, lsl],
                                    woT_sb[cc][:, 128 * oc:128 * (oc + 1)],
                                    relu_sb[cc][:, lsl],
                                    start=(cc == 0), stop=(cc == 1),
                                )
                            # alternate ACT/DVE so the final evictions don't
                            # serialize on one engine
                            if oc % 2:
                                nc.vector.scalar_tensor_tensor(
                                    o_sb[:, oc, lsl], ps_o[:, lsl], 0.0,
                                    recip[:, lsl], op0=Alu.bypass, op1=Alu.bypass,
                                )
                            else:
                                nc.scalar.activation(o_sb[:, oc, lsl], ps_o[:, lsl],
                                                     Act.Copy)
                            # oc-pair copies at half granularity: early issue
                            # overlaps the remaining evictions without filling
                            # the drain path with per-quarter issue overhead
                            if oc % 2 and l1 % 256 == 0:
                                nc.sync.dma_start(
                                    out=out_d.ap()[:, oc - 1:oc + 1,
                                                   c0 + l1 - 256: c0 + l1],
                                    in_=o_sb[:, oc - 1:oc + 1, l1 - 256:l1],
                                )

                pending = []
                for b in range(BPH):
                    if b == 0:
                        if h == 0:
                            k_group(0)
                        k_group(1)
                        q_group(0)
                        vT_group(range(0, 8) if h == 0 else range(4, 8))
                    else:
                        k_group(b + 1)
                        q_group(b)
                        vT_group(range(4 * b + 4, 4 * b + 8))
                    # prefetch the next half's x stream while this half still
                    # computes — its tiles are fresh buffers (xpool bufs=2)
                    if h + 1 < NBH and b == 2:
                        for p in range(2):
                            _x_dma(h + 1, p, 256, 1536)
                    elif h + 1 < NBH and b == 3:
                        for p in range(2):
                            _x_dma(h + 1, p, 1536, 2560)
                    pending.append(emit_energy(h, b))
                    if len(pending) > 1:
                        opending.append(emit_tail_a(pending.pop(0)))
                    if len(opending) > 1:
                        emit_tail_b(opending.pop(0))
                # the AV/relu of this half's final block must flush before the
                # next half's projections overwrite vT; its outproj (and any
                # older pending outproj) needs only sbo tiles + weights, so it
                # slides into the next half's stream — except at the very end,
                # where the final block takes the fused low-latency path
                if h < NBH - 1:
                    opending.append(emit_tail_a(pending.pop(0)))
                else:
                    while opending:
                        emit_tail_b(opending.pop(0))
                    emit_tail_last(pending.pop(0))
                prev_vT = vT_sb
                if h == 0:
                    # stash the k halo overlap for the next half (SBUF->SBUF
                    # DMA, off-engine); half1's block 0 reads it directly
                    kh_prev = []
                    for cc in range(2):
                        tkh = qkv.tile([128, 512], F16, tag=f"kh{cc}",
                                       name=f"kh{cc}")
                        nc.sync.dma_start(out=tkh,
                                          in_=k_sb[cc][:, LH:LH + 512])
                        kh_prev.append(tkh)
    nc.compile()
    return nc


_NC_CACHE = {}


def _get_nc():
    if "nc" not in _NC_CACHE:
        _NC_CACHE["nc"] = _build_program()
    return _NC_CACHE["nc"]


def _f8():
    try:
        import ml_dtypes
        return ml_dtypes.float8_e4m3
    except ImportError:  # pragma: no cover
        import jax.numpy as jnp
        return jnp.float8_e4m3


def _split8(a):
    f8 = _f8()
    hi = np.asarray(a, np.float32).astype(f8)
    lo = (np.asarray(a, np.float32) - hi.astype(np.float32)).astype(f8)
    return hi, lo


def make_in_maps(x1, mask, Wq, bq, Wk, bk, Wv, bv, Wo, bo):
    x1 = np.asarray(x1, dtype=np.float32).reshape(CIN, L)

    def _pairs(w):
        # (C_out=256, C_in=512) -> [128, 2(hl), 2(p), 2(j), C] DoubleRow pair
        # layout with global c_in = 128*(2p+j) + c_in_within
        ws = np.asarray(w, np.float32) * WSCALE
        hi, lo = _split8(ws.T)          # (512, 256) each
        def lay(a):
            return a.reshape(2, 2, 128, C).transpose(2, 0, 1, 3)
        return np.ascontiguousarray(np.stack([lay(hi), lay(lo)], axis=1))

    wq8 = _pairs(Wq)
    wk8 = _pairs(Wk)
    wv8 = _pairs(Wv)
    woT = (np.asarray(Wo, np.float32).T / WSCALE).astype(np.float16)
    # fp8 hi/lo of 16*Wo^T in [c_within, hl, cc, c_out] layout (the 16*64
    # product scaling is undone by the 2^-10 eviction scale)
    wo_hi, wo_lo = _split8(np.asarray(Wo, np.float32).T * 16.0)
    def _wo_lay(a):
        return a.reshape(2, 128, CIN).transpose(1, 0, 2)
    wo8 = np.ascontiguousarray(
        np.stack([_wo_lay(wo_hi), _wo_lay(wo_lo)], axis=1))

    cb32 = np.zeros((128, 264), np.float32)
    cb32[:, 0:2] = (np.asarray(bq, np.float32) * WSCALE).reshape(2, 128).T
    cb32[:, 2:4] = (np.asarray(bk, np.float32) * WSCALE).reshape(2, 128).T
    # padf/padl cols 4:8 filled per core below
    cb32[:, 8:264] = np.broadcast_to(
        (np.asarray(bv, np.float32) * WSCALE).reshape(1, C), (128, C))

    cb16 = np.zeros((128, 1152), np.float16)
    cb16[:, 0:128] = 1.0
    cb16[:, 128:640] = woT[0:128]
    cb16[:, 640:1152] = woT[128:256]

    in_maps = []
    for c in range(NCORES):
        g0 = LC * c - HALF
        x1h = np.zeros((CIN, HALO), np.float32)
        s0, s1 = max(g0, 0), min(g0 + HALO, L)
        x1h[:, s0 - g0:s1 - g0] = x1[:, s0:s1]
        xh, xl = _split8(x1h)
        # [p, c_within, hl, j, pos] with global channel 128*(2p+j) + c_within
        xhl = np.stack([xh.reshape(2, 2, 128, HALO),
                        xl.reshape(2, 2, 128, HALO)], axis=0)
        xhl = np.ascontiguousarray(xhl.transpose(1, 3, 0, 2, 4))
        cb = cb32.copy()
        cb[:, 4:6] = 0.0 if c == 0 else 1.0
        cb[:, 6:8] = 0.0 if c == NCORES - 1 else 1.0
        m = {
            "xhl": xhl,
            "wq": wq8, "wk": wk8, "wv": wv8, "wo8": wo8,
            "cb32": cb, "cb16": cb16,
        }
        in_maps.append(m)
    return in_maps


def postprocess(results, mask, bo):
    # per-core out is [128, 4, LC] c-major; channel g = 128*oc + c
    cols = np.concatenate(
        [np.asarray(results[c]["out"], np.float32).transpose(1, 0, 2)
         .reshape(CIN, LC) for c in range(NCORES)], axis=1)
    out = cols[None] + np.asarray(bo, np.float32)[None, :, None]
    return (out * np.asarray(mask, np.float32)).astype(np.float32)


def kernel(x1, x2, mask, Wq, bq, Wk, bk, Wv, bv, Wo, bo, **_unused):
    from concourse.bass_utils import run_bass_kernel_spmd

    nc = _get_nc()
    in_maps = make_in_maps(x1, mask, Wq, bq, Wk, bk, Wv, bv, Wo, bo)
    res = run_bass_kernel_spmd(nc, in_maps, core_ids=list(range(NCORES)))
    return postprocess(res.results, mask, bo)
